# revision 48
# baseline (speedup 1.0000x reference)
"""Bass/Tile kernel for nn_BinaryClassifierChain on 8 trn2 cores.

Math (per reference.py):
  wc   = softmax(word_class_features, axis=0)            # over batch dim
  base = concat([features, wc], -1)                      # [B, W, 1088]
  L    = base @ W[:, :1088].T + b                        # [B, W, 32]
  chain: p_i = sigmoid(L_i + sum_{j<i} Wbin[i, j] p_j)   # Wbin = W[:, 1088:]

Sharding: data-parallel over the words dim (1024 = 8 x 128); softmax
couples the batch dim, which stays whole per shard.

v7 design notes (prior versions measured on HW):
  - Tiny weight transforms (W^T chunks, replicated Wbin, block-diag wc
    weights, bias column) precomputed host-side; on-chip prep is just 4
    small loads.  Kills the v5/v6 startup cascade (wbf cast -> 9 serial
    DMA transposes -> softmax at t=50us).
  - wc loaded with partition=batch (64 fat descriptors -- small-desc
    DMAs starve against the 16 SWDGE feature streams, fat ones do not),
    then PE-transposed to [w, c, b] right after group 0.
  - Softmax in [w, c, b]; batch-pair DMA transposes (idle sync engine)
    give wctP [(c,b%2), pair, w]; per-group wc term = 2 pair matmuls
    with block-diag wcd2, summed with the corner-turn psum into L.
  - Chain: batches 0-31 on DVE interleaved into groups 8-15; tail is
    q2 (DVE) || q3 (gpsimd muls, DVE reduces).  corr is bf16
    (allow_low_precision) for the DVE all-16-bit fast path.
  - Store: ACT casts P quarters to f32, HWDGE stores.
"""

import sys

sys.path.insert(0, "/opt/trn_rl_repo")

import ml_dtypes
import numpy as np
import orjson

import concourse.bass as bass
import concourse.mybir as mybir
import concourse.tile as tile
from concourse import masks
from concourse.bass_utils import run_bass_kernel_spmd

F32 = mybir.dt.float32
BF16 = mybir.dt.bfloat16
AF = mybir.ActivationFunctionType
ALU = mybir.AluOpType
AX = mybir.AxisListType
NPBF16 = ml_dtypes.bfloat16

B = 64          # batch
NWALL = 1024    # total words
NCORES = 8
NW = NWALL // NCORES  # 128 words per core
D = 1024        # embed dim
C = 64          # word classes
NB = 32         # bin features
DIN = D + C + NB  # 1120
GRP = 4         # batches per matmul group (4 * 128 words = 512 tokens)
NGRP = B // GRP


def _split_multiwait_json(raw: bytes) -> bytes:
    """walrus in this container only accepts 1 sync-wait per most
    instructions; Tile's final drain (and some others) carry several.
    Move extras onto preceding EventSemaphore carriers (2 waits each) on
    the same engine."""
    bir = orjson.loads(raw)
    for fn in bir["functions"]:
        for blk in fn["blocks"]:
            out = []
            for ins in blk["instructions"]:
                si = ins.get("sync_info")
                waits = (si or {}).get("on_wait") or []
                if len(waits) > 1:
                    extra = waits[:-1]
                    for k in range(0, len(extra), 2):
                        out.append(
                            {
                                "debug": ins.get("debug", 0),
                                "engine": ins["engine"],
                                "ins": [],
                                "outs": [],
                                "name": f"{ins['name']}_sw{k}",
                                "opcode": "EventSemaphore",
                                "sync_info": {
                                    "on_update": [],
                                    "on_wait": extra[k : k + 2],
                                },
                            }
                        )
                    si["on_wait"] = [waits[-1]]
                out.append(ins)
            blk["instructions"] = out
    return orjson.dumps(bir)


def _even(i: int) -> int:
    return i + (i & 1)


def build_program():
    nc = bass.Bass("TRN2", target_bir_lowering=False, debug=False)

    feat = nc.dram_tensor("feat", [B, NW, D], F32, kind="ExternalInput")
    wc = nc.dram_tensor("wc", [B, NW, C], F32, kind="ExternalInput")
    wtr9_d = nc.dram_tensor("wtr9", [128, 9, NB], BF16, kind="ExternalInput")
    wrepb_d = nc.dram_tensor("wrepb", [128, NB * NB], BF16, kind="ExternalInput")
    wcd2_d = nc.dram_tensor("wcd2", [128, 64], BF16, kind="ExternalInput")
    # batch-block-diag Wbin cross-block weights: blkd[:, q, b*nj+j] row
    # (b,i) -> Wbin[8(q+1)+j, 8q+i] for the 16-batch chain transposes
    blkd_d = nc.dram_tensor("blkd", [128, 3, 384], BF16, kind="ExternalInput")
    bsb_d = nc.dram_tensor("bsb", [NB, 1], F32, kind="ExternalInput")
    out = nc.dram_tensor("out", [B, NW, NB], F32, kind="ExternalOutput")

    with tile.TileContext(nc) as tc:
        with (
            tc.tile_pool(name="const", bufs=1) as constp,
            tc.tile_pool(name="x2", bufs=4) as x2p,
            tc.tile_pool(name="xt", bufs=2) as xtp,
            tc.tile_pool(name="blt", bufs=6) as bltp,
            tc.tile_pool(name="lp", bufs=1) as lpp,
            tc.tile_pool(name="pf", bufs=2) as pfp,
            tc.tile_pool(name="tp", bufs=2, space="PSUM") as tpp,
            tc.tile_pool(name="mmps", bufs=2, space="PSUM") as mmpsp,
            tc.tile_pool(name="petps", bufs=1, space="PSUM") as petpsp,
            tc.tile_pool(name="wcps", bufs=1, space="PSUM") as wcpsp,
        ):
            # ---------------- constants (host-precomputed) ----------------
            ident = constp.tile([128, 128], BF16)
            masks.make_identity(nc, ident[:])
            identf = constp.tile([NB, NB], F32)
            masks.make_identity(nc, identf[:])


            wtr = constp.tile([128, 9, NB], BF16)
            nc.sync.dma_start(wtr[:], wtr9_d.ap())
            wrepb = constp.tile([128, NB * NB], BF16)
            nc.sync.dma_start(wrepb[:], wrepb_d.ap())
            wcd2 = constp.tile([128, 64], BF16)
            nc.sync.dma_start(wcd2[:], wcd2_d.ap())
            blkd = constp.tile([128, 3, 384], BF16)
            nc.sync.dma_start(blkd[:], blkd_d.ap())
            b_sb = constp.tile([NB, 1], F32)
            nc.sync.dma_start(b_sb[:], bsb_d.ap())

            # wc raw, partition = batch (fat descriptors, fast even while
            # the 16 feature streams run)
            wcraw = constp.tile([B, NW, C], F32)
            nc.sync.dma_start(wcraw[:], wc.ap())

            # softmax(wc) in pair-contiguous layout [w, pair, c, b%2]
            wcn3 = constp.tile([128, B // 2, C, 2], BF16)
            wctP = constp.tile([128, B // 2, NW], BF16)  # [(c,b%2), pair, w]

            # chain state
            L = lpp.tile([128, B, NB], F32)
            P = lpp.tile([128, B, NB], BF16)
            tmp0 = lpp.tile([128, 32, NB], BF16)
            tmp2 = lpp.tile([128, 16, NB], BF16)
            tmpg = lpp.tile([128, 16, NB], BF16)
            cor0 = lpp.tile([128, 32], BF16)
            cor2 = lpp.tile([128, 16], BF16)
            cor3 = lpp.tile([128, 16], BF16)
            nc.vector.memset(P[:], 0.0)

            wcs2 = constp.tile([128, C, B], BF16)   # raw wc, [w, c, b]
            ex = lpp.tile([128, C, B], F32)

            # ---------------- helpers ----------------
            x2_tiles = []

            def emit_load(g):
                b0 = g * GRP
                x2 = x2p.tile([128, GRP, D], BF16, tag="x2")
                nc.gpsimd.dma_start(
                    x2[:], feat.ap()[b0 : b0 + GRP, :, :].rearrange("b p d -> p b d")
                )
                x2_tiles.append(x2)

            blts = {}

            def body_A(g):
                x2 = x2_tiles[g]
                xts = xtp.tile([128, 8, GRP * 128], BF16, tag="xt")
                for kh in range(4):
                    pt = tpp.tile([128, 2, GRP * 128], BF16, tag="xtps")
                    for kk in range(2):
                        k = kh * 2 + kk
                        for bi in range(GRP):
                            nc.tensor.transpose(
                                pt[:, kk, bi * 128 : (bi + 1) * 128],
                                x2[:, bi, k * 128 : (k + 1) * 128],
                                ident[:],
                            )
                    if g % 2 == 0:
                        nc.vector.tensor_copy(xts[:, kh * 2 : kh * 2 + 2, :], pt[:])
                    else:
                        nc.scalar.copy(xts[:, kh * 2 : kh * 2 + 2, :], pt[:])
                ps = mmpsp.tile([NB, 512], F32, tag="mm")
                for k in range(8):
                    nc.tensor.matmul(
                        ps[:], wtr[:, k, :], xts[:, k, :],
                        start=(k == 0), stop=(k == 7),
                    )
                blt = bltp.tile([NB, 512], F32, tag="blt")
                nc.scalar.activation(
                    blt[:], ps[:], AF.Identity, bias=b_sb[:, 0:1], scale=1.0
                )
                blts[g] = blt

            wcb16 = constp.tile([B, NW, C], BF16)

            def emit_precast():
                # ACT casts raw wc to bf16 so the PE transposes are bf16
                nc.scalar.copy(wcb16[:], wcraw[:])

            def emit_wcT(t):
                # PE-transpose raw wc [b, w, c] -> wcs2 [w, c, b]
                wp = wcpsp.tile([128, 16, B], BF16, tag="wct")
                for j in range(16):
                    c = t * 16 + j
                    nc.tensor.transpose(
                        wp[:, j, :], wcb16[:, :, c], ident[0:B, 0:B]
                    )
                nc.vector.tensor_copy(wcs2[:, t * 16 : t * 16 + 16, :], wp[:])

            def emit_softmax():
                nc.scalar.activation(ex[:], wcs2[:], AF.Exp)
                acc = lpp.tile([128, C, B // 2], F32)
                nc.vector.tensor_add(
                    acc[:], ex[:, :, 0 : B // 2], ex[:, :, B // 2 : B]
                )
                h = B // 4
                while h >= 1:
                    nc.vector.tensor_add(
                        acc[:, :, 0:h], acc[:, :, 0:h], acc[:, :, h : 2 * h]
                    )
                    h //= 2
                rec = lpp.tile([128, C], F32)
                nc.vector.reciprocal(rec[:], acc[:, :, 0])
                for p in range(B // 2):
                    nc.vector.tensor_tensor(
                        wcn3[:, p, :, :],
                        ex[:, :, 2 * p : 2 * p + 2],
                        rec[:].unsqueeze(2).broadcast_to([128, C, 2]),
                        op=ALU.mult,
                    )

            def emit_wctP(blk):
                # PE-transpose 8 batch-pairs [128w, (c,b%2)] -> wctP
                wp = wcpsp.tile([128, 8, NW], BF16, tag="wcpt")
                for j in range(8):
                    p = blk * 8 + j
                    nc.tensor.transpose(
                        wp[:, j, :], wcn3[:, p, :, :], ident[:]
                    )
                nc.vector.tensor_copy(wctP[:, blk * 8 : (blk + 1) * 8, :], wp[:])

            def body_B(g):
                blt = blts[g]
                ptc = petpsp.tile([128, GRP, NB], F32, tag="pet")
                for q in range(GRP):
                    nc.tensor.transpose(
                        ptc[:, q, :], blt[:, q * 128 : (q + 1) * 128], identf[:]
                    )
                wcp = petpsp.tile([128, GRP, NB], F32, tag="wcp")
                for h in range(2):
                    nc.tensor.matmul(
                        wcp[:, 2 * h : 2 * h + 2, :], wctP[:, 2 * g + h, :], wcd2[:],
                        start=True, stop=True,
                    )
                wcsb = bltp.tile([128, GRP, NB], F32, tag="wcsb")
                nc.scalar.copy(wcsb[:], wcp[:])
                b0 = g * GRP
                nc.vector.scalar_tensor_tensor(
                    L[:, b0 : b0 + GRP, :], ptc[:], 1.0, wcsb[:],
                    op0=ALU.mult, op1=ALU.add,
                )

            # staging tiles for the chain cross-block transposes
            stg0 = lpp.tile([128, 16, 8], BF16)
            pts0 = lpp.tile([128, 128], BF16)
            stg1 = lpp.tile([128, 16, 8], BF16)
            pts1 = lpp.tile([128, 128], BF16)
            stg2 = lpp.tile([128, 16, 8], BF16)
            pts2 = lpp.tile([128, 128], BF16)
            stg3 = lpp.tile([128, 16, 8], BF16)
            pts3 = lpp.tile([128, 128], BF16)
            stages = {
                0: (stg0, pts0),
                16: (stg1, pts1),
                32: (stg2, pts2),
                48: (stg3, pts3),
            }

            def chain_cross(q, bs0):
                # add block q's contribution to all later bins of batches
                # [bs0, bs0+16): transpose P-block on PE, one matmul with
                # the block-diag weights, DVE-add into L
                stage, ptsb = stages[bs0]
                bs = slice(bs0, bs0 + 16)
                nj = NB - 8 * (q + 1)
                nc.vector.tensor_copy(stage[:], P[:, bs, 8 * q : 8 * q + 8])
                ptps = wcpsp.tile([128, NW], BF16, tag="wcpt")
                nc.tensor.transpose(ptps[:], stage[:], ident[:])
                nc.vector.tensor_copy(ptsb[:], ptps[:])
                cps = mmpsp.tile([128, 16, nj], F32, tag="mm")
                nc.tensor.matmul(
                    cps[:], ptsb[:], blkd[:, q, 0 : 16 * nj],
                    start=True, stop=True,
                )
                nc.vector.tensor_add(
                    L[:, bs, 8 * (q + 1) : NB],
                    L[:, bs, 8 * (q + 1) : NB],
                    cps[:],
                )

            def chain_step(i, bs, tmp, cor, mul_eng):
                # within-block correction only (cross-block arrives via
                # chain_cross); block-local width, rounded even
                nb_ = bs.stop - bs.start
                q0 = 8 * (i // 8)
                ie = _even(i - q0)
                if ie > 0:
                    wrow = wrepb[:, i * NB + q0 : i * NB + q0 + ie]
                    mul_eng.tensor_tensor(
                        tmp[:, 0:nb_, 0:ie],
                        P[:, bs, q0 : q0 + ie],
                        wrow.unsqueeze(1).broadcast_to([128, nb_, ie]),
                        op=ALU.mult,
                    )
                    nc.vector.tensor_reduce(
                        cor[:, 0:nb_], tmp[:, 0:nb_, 0:ie], axis=AX.X, op=ALU.add
                    )
                    nc.vector.scalar_tensor_tensor(
                        L[:, bs, i], cor[:, 0:nb_], 1.0, L[:, bs, i],
                        op0=ALU.mult, op1=ALU.add,
                    )
                nc.scalar.activation(P[:, bs, i], L[:, bs, i], AF.Sigmoid)

            def emit_store(bq):
                pf = pfp.tile([128, 16, NB], F32, tag="pf")
                nc.scalar.copy(pf[:], P[:, bq : bq + 16, :])
                nc.sync.dma_start(
                    out.ap()[bq : bq + 16, :, :].rearrange("b p i -> p b i"), pf[:]
                )

            # ---------------- emission schedule ----------------
            for g in range(NGRP):
                emit_load(g)

            with nc.allow_low_precision(reason="bf16 chain corr, products ~1e-1"):
                for g in range(NGRP):
                    body_A(g)
                    if g == 0:
                        emit_precast()
                    if g in (1, 2):
                        emit_wcT(2 * (g - 1))
                        emit_wcT(2 * (g - 1) + 1)
                    if g == 3:
                        emit_softmax()
                    if 4 <= g <= 7:
                        emit_wctP(g - 4)
                    if g == 5:
                        for gg in range(5):
                            body_B(gg)
                    if g >= 6:
                        body_B(g - 1)
                    if g >= 8:
                        b0_ = 4 * (g - 8)
                        if b0_ in (8, 16, 24):
                            chain_cross(b0_ // 8 - 1, 0)
                            chain_cross(b0_ // 8 - 1, 16)
                        for i in range(b0_, b0_ + 4):
                            chain_step(i, slice(0, 32), tmp0, cor0, nc.vector)
                body_B(15)

                emit_store(0)
                emit_store(16)

                # tail: q2 (b32-47) on DVE || q3 (b48-63) gpsimd muls,
                # blocked: cross-block terms via PE every 8 bins
                for i in range(NB):
                    bs2, bs3 = slice(32, 48), slice(48, 64)
                    if i in (8, 16, 24):
                        chain_cross(i // 8 - 1, 32)
                        chain_cross(i // 8 - 1, 48)
                    q0 = 8 * (i // 8)
                    ie = _even(i - q0)
                    if ie > 0:
                        wrow = wrepb[:, i * NB + q0 : i * NB + q0 + ie]
                        nc.vector.tensor_tensor(
                            tmp2[:, :, 0:ie], P[:, bs2, q0 : q0 + ie],
                            wrow.unsqueeze(1).broadcast_to([128, 16, ie]),
                            op=ALU.mult,
                        )
                        nc.gpsimd.tensor_tensor(
                            tmpg[:, :, 0:ie], P[:, bs3, q0 : q0 + ie],
                            wrow.unsqueeze(1).broadcast_to([128, 16, ie]),
                            op=ALU.mult,
                        )
                        nc.vector.tensor_reduce(
                            cor2[:], tmp2[:, :, 0:ie], axis=AX.X, op=ALU.add
                        )
                        nc.vector.scalar_tensor_tensor(
                            L[:, bs2, i], cor2[:], 1.0, L[:, bs2, i],
                            op0=ALU.mult, op1=ALU.add,
                        )
                        nc.vector.tensor_reduce(
                            cor3[:], tmpg[:, :, 0:ie], axis=AX.X, op=ALU.add
                        )
                        nc.gpsimd.tensor_tensor(
                            L[:, bs3, i], cor3[:], L[:, bs3, i], op=ALU.add
                        )
                    nc.scalar.activation(P[:, bs2, i], L[:, bs2, i], AF.Sigmoid)
                    nc.scalar.activation(P[:, bs3, i], L[:, bs3, i], AF.Sigmoid)

                emit_store(32)
                emit_store(48)

    orig = nc.to_json_bytes
    nc.to_json_bytes = lambda: _split_multiwait_json(orig())
    return nc


_PROG = None


def _get_prog():
    global _PROG
    if _PROG is None:
        _PROG = build_program()
    return _PROG


def _host_weights(W, b):
    wpad = np.zeros((NB, 1152), np.float32)
    wpad[:, 0:DIN] = W
    # wtr9[r, k, j] = W[j, 128k + r]
    wtr9 = np.ascontiguousarray(
        wpad.reshape(NB, 9, 128).transpose(2, 1, 0)
    ).astype(NPBF16)
    wrepb = np.ascontiguousarray(
        np.tile(W[:, D + C : DIN].reshape(1, -1), (128, 1))
    ).astype(NPBF16)
    # block-diag wc weights in (c, b%2)-interleaved row order:
    # wcd2[2c + bl, bl*32 + j] = W[j, D + c]
    wcd2 = np.zeros((128, 64), np.float32)
    wcW = W[:, D : D + C]  # [j, c]
    for bl in range(2):
        wcd2[bl::2, bl * NB : (bl + 1) * NB] = wcW.T
    wcd2 = wcd2.astype(NPBF16)
    # chain cross-block weights: blkd[b*8+i, q, b*nj+j] = Wbin[8(q+1)+j, 8q+i]
    wbin = W[:, D + C : DIN]  # [j, i]
    blkd = np.zeros((128, 3, 384), np.float32)
    for q in range(3):
        nj = NB - 8 * (q + 1)
        blk = wbin[8 * (q + 1) : NB, 8 * q : 8 * q + 8].T  # [8 i, nj]
        for bb in range(16):
            blkd[bb * 8 : bb * 8 + 8, q, bb * nj : (bb + 1) * nj] = blk
    blkd = blkd.astype(NPBF16)
    bsb = np.ascontiguousarray(b.reshape(NB, 1)).astype(np.float32)
    return wtr9, wrepb, wcd2, blkd, bsb


def kernel(features, word_class_features, W, b, trace=False, tmpdir=None):
    features = np.ascontiguousarray(features, dtype=np.float32)
    word_class_features = np.ascontiguousarray(word_class_features, dtype=np.float32)
    W = np.ascontiguousarray(W, dtype=np.float32)
    b = np.ascontiguousarray(b, dtype=np.float32)
    wtr9, wrepb, wcd2, blkd, bsb = _host_weights(W, b)

    nc = _get_prog()
    in_maps = []
    for c in range(NCORES):
        sl = slice(c * NW, (c + 1) * NW)
        in_maps.append(
            {
                "feat": np.ascontiguousarray(features[:, sl, :]),
                "wc": np.ascontiguousarray(word_class_features[:, sl, :]),
                "wtr9": wtr9,
                "wrepb": wrepb,
                "wcd2": wcd2,
                "blkd": blkd,
                "bsb": bsb,
            }
        )
    res = run_bass_kernel_spmd(
        nc, in_maps, core_ids=list(range(NCORES)), trace=trace, tmpdir=tmpdir
    )
    outp = np.concatenate([res.results[c]["out"] for c in range(NCORES)], axis=1)
    kernel._last_result = res
    return outp


# revision 52
# speedup vs baseline: 1.1611x; 1.1611x over previous
"""Bass/Tile kernel for nn_BinaryClassifierChain on 8 trn2 cores.

Math (per reference.py):
  wc   = softmax(word_class_features, axis=0)            # over batch dim
  base = concat([features, wc], -1)                      # [B, W, 1088]
  L    = base @ W[:, :1088].T + b                        # [B, W, 32]
  chain: p_i = sigmoid(L_i + sum_{j<i} Wbin[i, j] p_j)   # Wbin = W[:, 1088:]

Sharding: data-parallel over the words dim (1024 = 8 x 128); softmax
couples the batch dim, which stays whole per shard.

v7 design notes (prior versions measured on HW):
  - Tiny weight transforms (W^T chunks, replicated Wbin, block-diag wc
    weights, bias column) precomputed host-side; on-chip prep is just 4
    small loads.  Kills the v5/v6 startup cascade (wbf cast -> 9 serial
    DMA transposes -> softmax at t=50us).
  - wc loaded with partition=batch (64 fat descriptors -- small-desc
    DMAs starve against the 16 SWDGE feature streams, fat ones do not),
    then PE-transposed to [w, c, b] right after group 0.
  - Softmax in [w, c, b]; batch-pair DMA transposes (idle sync engine)
    give wctP [(c,b%2), pair, w]; per-group wc term = 2 pair matmuls
    with block-diag wcd2, summed with the corner-turn psum into L.
  - Chain: batches 0-31 on DVE interleaved into groups 8-15; tail is
    q2 (DVE) || q3 (gpsimd muls, DVE reduces).  corr is bf16
    (allow_low_precision) for the DVE all-16-bit fast path.
  - Store: ACT casts P quarters to f32, HWDGE stores.
"""

import sys

sys.path.insert(0, "/opt/trn_rl_repo")

import ml_dtypes
import numpy as np
import orjson

import concourse.bass as bass
import concourse.mybir as mybir
import concourse.tile as tile
from concourse import masks
from concourse.bass_utils import run_bass_kernel_spmd

F32 = mybir.dt.float32
BF16 = mybir.dt.bfloat16
AF = mybir.ActivationFunctionType
ALU = mybir.AluOpType
AX = mybir.AxisListType
NPBF16 = ml_dtypes.bfloat16

B = 64          # batch
NWALL = 1024    # total words
NCORES = 8
NW = NWALL // NCORES  # 128 words per core
D = 1024        # embed dim
C = 64          # word classes
NB = 32         # bin features
DIN = D + C + NB  # 1120
GRP = 4         # batches per matmul group (4 * 128 words = 512 tokens)
NGRP = B // GRP


def _split_multiwait_json(raw: bytes) -> bytes:
    """walrus in this container only accepts 1 sync-wait per most
    instructions; Tile's final drain (and some others) carry several.
    Move extras onto preceding EventSemaphore carriers (2 waits each) on
    the same engine."""
    bir = orjson.loads(raw)
    for fn in bir["functions"]:
        for blk in fn["blocks"]:
            out = []
            for ins in blk["instructions"]:
                si = ins.get("sync_info")
                waits = (si or {}).get("on_wait") or []
                if len(waits) > 1:
                    extra = waits[:-1]
                    for k in range(0, len(extra), 2):
                        out.append(
                            {
                                "debug": ins.get("debug", 0),
                                "engine": ins["engine"],
                                "ins": [],
                                "outs": [],
                                "name": f"{ins['name']}_sw{k}",
                                "opcode": "EventSemaphore",
                                "sync_info": {
                                    "on_update": [],
                                    "on_wait": extra[k : k + 2],
                                },
                            }
                        )
                    si["on_wait"] = [waits[-1]]
                out.append(ins)
            blk["instructions"] = out
    return orjson.dumps(bir)


def _even(i: int) -> int:
    return i + (i & 1)


def build_program():
    nc = bass.Bass("TRN2", target_bir_lowering=False, debug=False)

    feat = nc.dram_tensor("feat", [B, NW, D], F32, kind="ExternalInput")
    wc = nc.dram_tensor("wc", [B, NW, C], F32, kind="ExternalInput")
    wtr9_d = nc.dram_tensor("wtr9", [128, 9, NB], BF16, kind="ExternalInput")
    wrepb_d = nc.dram_tensor("wrepb", [128, NB * NB], BF16, kind="ExternalInput")
    wcd2_d = nc.dram_tensor("wcd2", [128, 64], BF16, kind="ExternalInput")
    # batch-block-diag Wbin cross-block weights: blkd[:, q, b*nj+j] row
    # (b,i) -> Wbin[8(q+1)+j, 8q+i] for the 16-batch chain transposes
    blkd_d = nc.dram_tensor("blkd", [128, 3, 384], BF16, kind="ExternalInput")
    bsb_d = nc.dram_tensor("bsb", [NB, 1], F32, kind="ExternalInput")
    out = nc.dram_tensor("out", [B, NW, NB], F32, kind="ExternalOutput")

    with tile.TileContext(nc) as tc:
        with (
            tc.tile_pool(name="const", bufs=1) as constp,
            tc.tile_pool(name="x2", bufs=5) as x2p,
            tc.tile_pool(name="xt", bufs=2) as xtp,
            tc.tile_pool(name="blt", bufs=6) as bltp,
            tc.tile_pool(name="lp", bufs=1) as lpp,
            tc.tile_pool(name="pf", bufs=2) as pfp,
            tc.tile_pool(name="tp", bufs=3, space="PSUM") as tpp,
            tc.tile_pool(name="mmps", bufs=2, space="PSUM") as mmpsp,
            tc.tile_pool(name="petps", bufs=1, space="PSUM") as petpsp,
            tc.tile_pool(name="wcps", bufs=1, space="PSUM") as wcpsp,
        ):
            # ---------------- constants (host-precomputed) ----------------
            ident = constp.tile([128, 128], BF16)
            masks.make_identity(nc, ident[:])
            identf = constp.tile([NB, NB], F32)
            masks.make_identity(nc, identf[:])


            wtr = constp.tile([128, 9, NB], BF16)
            nc.sync.dma_start(wtr[:], wtr9_d.ap())
            wrepb = constp.tile([128, NB * NB], BF16)
            nc.sync.dma_start(wrepb[:], wrepb_d.ap())
            wcd2 = constp.tile([128, 64], BF16)
            nc.sync.dma_start(wcd2[:], wcd2_d.ap())
            blkd = constp.tile([128, 3, 384], BF16)
            nc.sync.dma_start(blkd[:], blkd_d.ap())
            b_sb = constp.tile([NB, 1], F32)
            nc.sync.dma_start(b_sb[:], bsb_d.ap())

            # wc raw, partition = batch (fat descriptors, fast even while
            # the 16 feature streams run)
            wcraw = constp.tile([B, NW, C], F32)
            nc.sync.dma_start(wcraw[:], wc.ap())

            # softmax(wc) in pair-contiguous layout [w, pair, c, b%2]
            wcn3 = constp.tile([128, B // 2, C, 2], BF16)
            wctP = constp.tile([128, B // 2, NW], BF16)  # [(c,b%2), pair, w]

            # chain state
            L = lpp.tile([128, B, NB], F32)
            P = lpp.tile([128, B, NB], BF16)
            tmp0 = lpp.tile([128, 32, NB], BF16)
            tmp2 = lpp.tile([128, 16, NB], BF16)
            tmpg = lpp.tile([128, 16, NB], BF16)
            cor0 = lpp.tile([128, 32], BF16)
            cor2 = lpp.tile([128, 16], BF16)
            cor3 = lpp.tile([128, 16], BF16)
            nc.vector.memset(P[:], 0.0)

            wcs2 = constp.tile([128, C, B], BF16)   # raw wc, [w, c, b]
            ex = lpp.tile([128, C, B], F32)

            # ---------------- helpers ----------------
            x2_tiles = []

            def emit_load(g):
                b0 = g * GRP
                x2 = x2p.tile([128, GRP, D], BF16, tag="x2")
                if g == 0:
                    # split the first load so group 0's transposes can
                    # begin on the first half ~3us earlier
                    nc.gpsimd.dma_start(
                        x2[:, 0:2, :],
                        feat.ap()[b0 : b0 + 2, :, :].rearrange("b p d -> p b d"),
                    )
                    nc.gpsimd.dma_start(
                        x2[:, 2:4, :],
                        feat.ap()[b0 + 2 : b0 + 4, :, :].rearrange("b p d -> p b d"),
                    )
                else:
                    nc.gpsimd.dma_start(
                        x2[:],
                        feat.ap()[b0 : b0 + GRP, :, :].rearrange("b p d -> p b d"),
                    )
                x2_tiles.append(x2)

            blts = {}

            def body_A(g):
                x2 = x2_tiles[g]
                xts = xtp.tile([128, 8, GRP * 128], BF16, tag="xt")
                for kh in range(4):
                    pt = tpp.tile([128, 2, GRP * 128], BF16, tag="xtps")
                    for kk in range(2):
                        k = kh * 2 + kk
                        for bi in range(GRP):
                            nc.tensor.transpose(
                                pt[:, kk, bi * 128 : (bi + 1) * 128],
                                x2[:, bi, k * 128 : (k + 1) * 128],
                                ident[:],
                            )
                    if g % 2 == 0:
                        nc.vector.tensor_copy(xts[:, kh * 2 : kh * 2 + 2, :], pt[:])
                    else:
                        nc.scalar.copy(xts[:, kh * 2 : kh * 2 + 2, :], pt[:])
                ps = mmpsp.tile([NB, 512], F32, tag="mm")
                for k in range(8):
                    nc.tensor.matmul(
                        ps[:], wtr[:, k, :], xts[:, k, :],
                        start=(k == 0), stop=(k == 7),
                    )
                blt = bltp.tile([NB, 512], F32, tag="blt")
                nc.scalar.activation(
                    blt[:], ps[:], AF.Identity, bias=b_sb[:, 0:1], scale=1.0
                )
                blts[g] = blt

            wcb16 = constp.tile([B, NW, C], BF16)

            def emit_precast():
                # ACT casts raw wc to bf16 so the PE transposes are bf16
                nc.scalar.copy(wcb16[:], wcraw[:])

            def emit_wcT(t):
                # PE-transpose raw wc [b, w, c] -> wcs2 [w, c, b]; psum
                # from the pet tag (temporally disjoint from body_B use)
                wp = petpsp.tile([128, 16, B], BF16, tag="pet")
                for j in range(16):
                    c = t * 16 + j
                    nc.tensor.transpose(
                        wp[:, j, :], wcb16[:, :, c], ident[0:B, 0:B]
                    )
                nc.vector.tensor_copy(wcs2[:, t * 16 : t * 16 + 16, :], wp[:])

            def emit_softmax():
                nc.scalar.activation(ex[:], wcs2[:], AF.Exp)
                acc = lpp.tile([128, C, B // 2], F32)
                nc.vector.tensor_add(
                    acc[:], ex[:, :, 0 : B // 2], ex[:, :, B // 2 : B]
                )
                h = B // 4
                while h >= 1:
                    nc.vector.tensor_add(
                        acc[:, :, 0:h], acc[:, :, 0:h], acc[:, :, h : 2 * h]
                    )
                    h //= 2
                rec = lpp.tile([128, C], F32)
                nc.vector.reciprocal(rec[:], acc[:, :, 0])
                for p in range(B // 2):
                    nc.vector.tensor_tensor(
                        wcn3[:, p, :, :],
                        ex[:, :, 2 * p : 2 * p + 2],
                        rec[:].unsqueeze(2).broadcast_to([128, C, 2]),
                        op=ALU.mult,
                    )

            def emit_wctP(blk):
                # PE-transpose 8 batch-pairs [128w, (c,b%2)] -> wctP
                wp = wcpsp.tile([128, 8, NW], BF16, tag="wcpt")
                for j in range(8):
                    p = blk * 8 + j
                    nc.tensor.transpose(
                        wp[:, j, :], wcn3[:, p, :, :], ident[:]
                    )
                nc.vector.tensor_copy(wctP[:, blk * 8 : (blk + 1) * 8, :], wp[:])

            def body_B(g):
                blt = blts[g]
                ptc = petpsp.tile([128, GRP, NB], F32, tag="pet")
                for q in range(GRP):
                    nc.tensor.transpose(
                        ptc[:, q, :], blt[:, q * 128 : (q + 1) * 128], identf[:]
                    )
                wcp = petpsp.tile([128, GRP, NB], F32, tag="wcp")
                for h in range(2):
                    nc.tensor.matmul(
                        wcp[:, 2 * h : 2 * h + 2, :], wctP[:, 2 * g + h, :], wcd2[:],
                        start=True, stop=True,
                    )
                wcsb = bltp.tile([128, GRP, NB], F32, tag="wcsb")
                nc.scalar.copy(wcsb[:], wcp[:])
                b0 = g * GRP
                nc.vector.scalar_tensor_tensor(
                    L[:, b0 : b0 + GRP, :], ptc[:], 1.0, wcsb[:],
                    op0=ALU.mult, op1=ALU.add,
                )

            # staging tiles for the chain cross-block transposes
            stg0 = lpp.tile([128, 16, 8], BF16)
            pts0 = lpp.tile([128, 128], BF16)
            stg1 = lpp.tile([128, 16, 8], BF16)
            pts1 = lpp.tile([128, 128], BF16)
            stg2 = lpp.tile([128, 16, 8], BF16)
            pts2 = lpp.tile([128, 128], BF16)
            stg3 = lpp.tile([128, 16, 8], BF16)
            pts3 = lpp.tile([128, 128], BF16)
            stages = {
                0: (stg0, pts0),
                16: (stg1, pts1),
                32: (stg2, pts2),
                48: (stg3, pts3),
            }

            def chain_cross(q, bs0):
                # add block q's contribution to all later bins of batches
                # [bs0, bs0+16): transpose P-block on PE, one matmul with
                # the block-diag weights, DVE-add into L
                stage, ptsb = stages[bs0]
                bs = slice(bs0, bs0 + 16)
                nj = NB - 8 * (q + 1)
                nc.vector.tensor_copy(stage[:], P[:, bs, 8 * q : 8 * q + 8])
                ptps = wcpsp.tile([128, NW], BF16, tag="wcpt")
                nc.tensor.transpose(ptps[:], stage[:], ident[:])
                nc.vector.tensor_copy(ptsb[:], ptps[:])
                cps = mmpsp.tile([128, 16, nj], F32, tag="mm")
                nc.tensor.matmul(
                    cps[:], ptsb[:], blkd[:, q, 0 : 16 * nj],
                    start=True, stop=True,
                )
                nc.vector.tensor_add(
                    L[:, bs, 8 * (q + 1) : NB],
                    L[:, bs, 8 * (q + 1) : NB],
                    cps[:],
                )

            def chain_step(i, bs, tmp, cor, mul_eng):
                # within-block correction only (cross-block arrives via
                # chain_cross); block-local width, rounded even
                nb_ = bs.stop - bs.start
                q0 = 8 * (i // 8)
                ie = _even(i - q0)
                if ie > 0:
                    wrow = wrepb[:, i * NB + q0 : i * NB + q0 + ie]
                    mul_eng.tensor_tensor(
                        tmp[:, 0:nb_, 0:ie],
                        P[:, bs, q0 : q0 + ie],
                        wrow.unsqueeze(1).broadcast_to([128, nb_, ie]),
                        op=ALU.mult,
                    )
                    nc.vector.tensor_reduce(
                        cor[:, 0:nb_], tmp[:, 0:nb_, 0:ie], axis=AX.X, op=ALU.add
                    )
                    nc.vector.scalar_tensor_tensor(
                        L[:, bs, i], cor[:, 0:nb_], 1.0, L[:, bs, i],
                        op0=ALU.mult, op1=ALU.add,
                    )
                nc.scalar.activation(P[:, bs, i], L[:, bs, i], AF.Sigmoid)

            def emit_store(bq):
                pf = pfp.tile([128, 16, NB], F32, tag="pf")
                nc.scalar.copy(pf[:], P[:, bq : bq + 16, :])
                nc.sync.dma_start(
                    out.ap()[bq : bq + 16, :, :].rearrange("b p i -> p b i"), pf[:]
                )

            # ---------------- emission schedule ----------------
            for g in range(NGRP):
                emit_load(g)

            with nc.allow_low_precision(reason="bf16 chain corr, products ~1e-1"):
                for g in range(NGRP):
                    body_A(g)
                    if g == 0:
                        emit_precast()
                    if g in (1, 2):
                        emit_wcT(2 * (g - 1))
                        emit_wcT(2 * (g - 1) + 1)
                    if g == 3:
                        emit_softmax()
                    if 4 <= g <= 7:
                        emit_wctP(g - 4)
                    if g == 5:
                        for gg in range(5):
                            body_B(gg)
                    if g >= 6:
                        body_B(g - 1)
                    if g >= 8:
                        b0_ = 4 * (g - 8)
                        if b0_ in (8, 16, 24):
                            chain_cross(b0_ // 8 - 1, 0)
                            chain_cross(b0_ // 8 - 1, 16)
                        for i in range(b0_, b0_ + 4):
                            chain_step(i, slice(0, 32), tmp0, cor0, nc.vector)
                body_B(15)

                emit_store(0)
                emit_store(16)

                # tail: q2 (b32-47) on DVE || q3 (b48-63) gpsimd muls,
                # blocked: cross-block terms via PE every 8 bins
                for i in range(NB):
                    bs2, bs3 = slice(32, 48), slice(48, 64)
                    if i in (8, 16, 24):
                        chain_cross(i // 8 - 1, 32)
                        chain_cross(i // 8 - 1, 48)
                    q0 = 8 * (i // 8)
                    ie = _even(i - q0)
                    if ie > 0:
                        wrow = wrepb[:, i * NB + q0 : i * NB + q0 + ie]
                        nc.vector.tensor_tensor(
                            tmp2[:, :, 0:ie], P[:, bs2, q0 : q0 + ie],
                            wrow.unsqueeze(1).broadcast_to([128, 16, ie]),
                            op=ALU.mult,
                        )
                        nc.gpsimd.tensor_tensor(
                            tmpg[:, :, 0:ie], P[:, bs3, q0 : q0 + ie],
                            wrow.unsqueeze(1).broadcast_to([128, 16, ie]),
                            op=ALU.mult,
                        )
                        nc.vector.tensor_reduce(
                            cor2[:], tmp2[:, :, 0:ie], axis=AX.X, op=ALU.add
                        )
                        nc.vector.scalar_tensor_tensor(
                            L[:, bs2, i], cor2[:], 1.0, L[:, bs2, i],
                            op0=ALU.mult, op1=ALU.add,
                        )
                        nc.vector.tensor_reduce(
                            cor3[:], tmpg[:, :, 0:ie], axis=AX.X, op=ALU.add
                        )
                        nc.gpsimd.tensor_tensor(
                            L[:, bs3, i], cor3[:], L[:, bs3, i], op=ALU.add
                        )
                    nc.scalar.activation(P[:, bs2, i], L[:, bs2, i], AF.Sigmoid)
                    nc.scalar.activation(P[:, bs3, i], L[:, bs3, i], AF.Sigmoid)

                emit_store(32)
                emit_store(48)

    orig = nc.to_json_bytes
    nc.to_json_bytes = lambda: _split_multiwait_json(orig())
    return nc


_PROG = None


def _get_prog():
    global _PROG
    if _PROG is None:
        _PROG = build_program()
    return _PROG


def _host_weights(W, b):
    wpad = np.zeros((NB, 1152), np.float32)
    wpad[:, 0:DIN] = W
    # wtr9[r, k, j] = W[j, 128k + r]
    wtr9 = np.ascontiguousarray(
        wpad.reshape(NB, 9, 128).transpose(2, 1, 0)
    ).astype(NPBF16)
    wrepb = np.ascontiguousarray(
        np.tile(W[:, D + C : DIN].reshape(1, -1), (128, 1))
    ).astype(NPBF16)
    # block-diag wc weights in (c, b%2)-interleaved row order:
    # wcd2[2c + bl, bl*32 + j] = W[j, D + c]
    wcd2 = np.zeros((128, 64), np.float32)
    wcW = W[:, D : D + C]  # [j, c]
    for bl in range(2):
        wcd2[bl::2, bl * NB : (bl + 1) * NB] = wcW.T
    wcd2 = wcd2.astype(NPBF16)
    # chain cross-block weights: blkd[b*8+i, q, b*nj+j] = Wbin[8(q+1)+j, 8q+i]
    wbin = W[:, D + C : DIN]  # [j, i]
    blkd = np.zeros((128, 3, 384), np.float32)
    for q in range(3):
        nj = NB - 8 * (q + 1)
        blk = wbin[8 * (q + 1) : NB, 8 * q : 8 * q + 8].T  # [8 i, nj]
        for bb in range(16):
            blkd[bb * 8 : bb * 8 + 8, q, bb * nj : (bb + 1) * nj] = blk
    blkd = blkd.astype(NPBF16)
    bsb = np.ascontiguousarray(b.reshape(NB, 1)).astype(np.float32)
    return wtr9, wrepb, wcd2, blkd, bsb


def kernel(features, word_class_features, W, b, trace=False, tmpdir=None):
    features = np.ascontiguousarray(features, dtype=np.float32)
    word_class_features = np.ascontiguousarray(word_class_features, dtype=np.float32)
    W = np.ascontiguousarray(W, dtype=np.float32)
    b = np.ascontiguousarray(b, dtype=np.float32)
    wtr9, wrepb, wcd2, blkd, bsb = _host_weights(W, b)

    nc = _get_prog()
    in_maps = []
    for c in range(NCORES):
        sl = slice(c * NW, (c + 1) * NW)
        in_maps.append(
            {
                "feat": np.ascontiguousarray(features[:, sl, :]),
                "wc": np.ascontiguousarray(word_class_features[:, sl, :]),
                "wtr9": wtr9,
                "wrepb": wrepb,
                "wcd2": wcd2,
                "blkd": blkd,
                "bsb": bsb,
            }
        )
    res = run_bass_kernel_spmd(
        nc, in_maps, core_ids=list(range(NCORES)), trace=trace, tmpdir=tmpdir
    )
    outp = np.concatenate([res.results[c]["out"] for c in range(NCORES)], axis=1)
    kernel._last_result = res
    return outp
